# revision 9
# baseline (speedup 1.0000x reference)
"""NVFP4 (E2M1, block-16) dequant matmul on 8 TRN2 NeuronCores — v5.

out[m, n] = sum_k (LUT[x[m,k]] * xs[m,k//16] * gx) * (LUT[w[n,k]] * ws[n,k//16] * gw) + bias[n]

Sharding: tensor-parallel along N: each of the 8 cores owns 1024 output
columns (weight/weight_scale/bias rows); x replicated.

v5 design (measured-fact driven):
  - DVE tensor_tensor at 2x bf16 is the only efficient multiplier
    (~2.3us/group); it runs every dequant multiply, pair-fused where
    possible against a stride-0-broadcast wsT. GpSimd never runs tensor
    ops (shared SBUF port would halve both engines).
  - DVE consumes dequant inputs faster than any single delivery path
    can供给, so three paths run in parallel, each with ~2us per-DMA
    receipt serialization per ring:
      * SWDGE cast-DMAs land 5 groups as bf16 (g0 halved + 4 singles),
      * ScalarE pair-casts 4 fp8 groups + chunk-casts the tail group,
      * 5 groups ship as raw bf16 from host on the HWDGE rings (HBM has
        ~10us of slack, fabric is the scarce resource).
  - Small x-side tensors ride SWDGE between weight loads (their receipt
    latency would otherwise stall the DVE queue head).
  - g0 chunk-granular at the head, g14 at the tail; per-half PSUM stop.

Host-side marshaling stays format-only (LUT decode + layout + dtype cast);
all reference arithmetic (scale multiplies, matmul, bias) is on device.
"""

import json
from contextlib import ExitStack

import ml_dtypes
import numpy as np

import concourse.bass as bass
import concourse.mybir as mybir
import concourse.tile as tile
from concourse.bass_utils import run_bass_kernel_spmd


def _split_multi_waits(m: dict) -> dict:
    """This walrus build allows at most one sync-wait command per instruction.
    Hoist extra waits into standalone EventSemaphore instructions issued just
    before the owning instruction on the same engine queue (semantically
    identical: the engine stalls in order)."""
    for fn in m["functions"]:
        for blk in fn["blocks"]:
            new = []
            ctr = 0
            for inst in blk["instructions"]:
                si = inst.get("sync_info")
                waits = (si or {}).get("on_wait") or []
                if len(waits) > 1:
                    for w in waits[:-1]:
                        new.append({
                            "debug": inst.get("debug", 0),
                            "engine": inst["engine"],
                            "ins": [],
                            "outs": [],
                            "name": f"{inst['name']}-hw{ctr}",
                            "opcode": "EventSemaphore",
                            "sync_info": {"on_update": [], "on_wait": [w]},
                        })
                        ctr += 1
                    si["on_wait"] = [waits[-1]]
                new.append(inst)
            blk["instructions"] = new
    return m


class _SplitWaitBass(bass.Bass):
    def to_json_bytes(self) -> bytes:
        m = json.loads(super().to_json_bytes())
        return json.dumps(_split_multi_waits(m)).encode()


BF16 = ml_dtypes.bfloat16
FP4_LUT = np.array(
    [0.0, 0.5, 1.0, 1.5, 2.0, 3.0, 4.0, 6.0,
     -0.0, -0.5, -1.0, -1.5, -2.0, -3.0, -4.0, -6.0],
    dtype=np.float32,
)

M, K, N = 64, 8192, 8192
NCORES = 8
NS = N // NCORES        # 1024 output columns per core
BLOCK = 16
B = K // BLOCK          # 512 scale blocks along K
P = 128                 # partitions
CHUNKS = K // P         # 64 K-chunks
CB = B // P             # 4 scale-chunk columns (c index)
J = BLOCK               # 16 j-groups (one group = CB chunks = 512 rows)
GW = CB * NS            # 4096 columns per weight group tile
NQ = NS                 # 1024 columns per chunk of a group tile

# Host weight layout (wv8p fp8 codes + wvbp bf16 values share this order):
#   pos 0:      g0        (SWDGE cast-DMA, halved, head)
#   pos 1-4:    g1,3,5,7  (SWDGE cast-DMA singles)
#   pos 5-8:    (2,4),(6,8)      A-pairs: fp8 slabs, ACT pair-cast
#   pos 9-12:   (9,11),(13,15)   B-pairs: shipped bf16, sync ring
#   pos 13:     g14       (fp8, ACT chunk-cast, tail)
#   pos 14-15:  (10,12)   B-pair: shipped bf16, scalar ring
NEWORD = (0, 1, 3, 5, 7, 2, 4, 6, 8, 9, 11, 13, 15, 14, 10, 12)
POS = {g: i for i, g in enumerate(NEWORD)}
S_SINGLES = (1, 3, 5, 7)
A_PAIRS = ((2, 4), (6, 8))
B_PAIRS_SYNC = ((9, 11), (13, 15))
B_PAIR_SCALAR = (10, 12)

_CACHE: dict = {}


def _build_program() -> bass.Bass:
    nc = _SplitWaitBass("TRN2", target_bir_lowering=False, debug=False,
                        num_devices=NCORES)
    dt = mybir.dt

    # fp8 codes for S/A/g0/g14 groups (positions 0-8, 13); bf16 values for
    # B groups (positions 9-12, 14-15). Shipped as two tensors.
    wv8p = nc.dram_tensor("wv8p", [P, 10 * GW], dt.float8e4,
                          kind="ExternalInput").ap()
    wvbp = nc.dram_tensor("wvbp", [P, 6 * GW], dt.bfloat16,
                          kind="ExternalInput").ap()
    wst = nc.dram_tensor("wst", [P, GW], dt.bfloat16,
                         kind="ExternalInput").ap()
    xvp = nc.dram_tensor("xvp", [P, CHUNKS * M], dt.bfloat16,
                         kind="ExternalInput").ap()
    xst = nc.dram_tensor("xst", [P, CB * M], dt.bfloat16,
                         kind="ExternalInput").ap()
    gs = nc.dram_tensor("gs", [P, 2], dt.float32, kind="ExternalInput").ap()
    bia = nc.dram_tensor("bia", [1, NS], dt.bfloat16, kind="ExternalInput").ap()
    out = nc.dram_tensor("out", [M, NS], dt.bfloat16, kind="ExternalOutput").ap()

    with tile.TileContext(nc) as tc, ExitStack() as ctx:
        const = ctx.enter_context(tc.tile_pool(name="const", bufs=1))
        w8pool = ctx.enter_context(tc.tile_pool(name="w8", bufs=1))
        whpool = ctx.enter_context(tc.tile_pool(name="wh", bufs=1))
        ppool = ctx.enter_context(tc.tile_pool(name="acc", bufs=1, space="PSUM"))

        # ---- tiles ----
        wsT = const.tile([P, GW], dt.bfloat16)
        xva = const.tile([P, CHUNKS * M], dt.bfloat16)
        gt = const.tile([P, 2], dt.float32)
        xsT = const.tile([P, CB * M], dt.bfloat16)
        bsb = const.tile([1, NS], dt.bfloat16)
        whS0 = whpool.tile([P, GW], dt.bfloat16, name="whS0")
        whS: dict = {}
        for g in S_SINGLES:
            whS[g] = whpool.tile([P, GW], dt.bfloat16, name=f"whS{g}")
        ptA: dict = {}
        for a, b in A_PAIRS:
            ptA[a] = whpool.tile([P, 2 * GW], dt.bfloat16, name=f"ptA{a}")
        ptB: dict = {}
        for a, b in B_PAIRS_SYNC + (B_PAIR_SCALAR,):
            ptB[a] = whpool.tile([P, 2 * GW], dt.bfloat16, name=f"ptB{a}")
        whA14 = whpool.tile([P, GW], dt.bfloat16, name="whA14")
        w8: dict = {}
        for a, b in A_PAIRS:
            w8[a] = w8pool.tile([P, 2 * GW], dt.float8e4, name=f"w8_{a}")
        w8[14] = w8pool.tile([P, GW], dt.float8e4, name="w8_14")

        # ---- DMAs ----
        # scalar/HWDGE ring: wsT (quarter then rest — gates every TT),
        # g14 fp8, B-pair (10,12) bf16, (out at the end).
        nc.scalar.dma_start(wsT[:, 0:NQ], wst[:, 0:NQ])
        nc.scalar.dma_start(wsT[:, NQ:GW], wst[:, NQ:GW])
        nc.scalar.dma_start(w8[14][:], wv8p[:, 9 * GW:10 * GW])
        nc.scalar.dma_start(ptB[10][:], wvbp[:, 4 * GW:6 * GW])

        # SWDGE ring: g0 halves (cast->bf16), tiny x tensors (cheap Q7
        # issues, keeps their receipt latency off the HWDGE rings), then
        # the four S singles.
        nc.gpsimd.dma_start(whS0[:, 0:2 * NQ], wv8p[:, 0:2 * NQ])
        nc.gpsimd.dma_start(whS0[:, 2 * NQ:GW], wv8p[:, 2 * NQ:GW])
        nc.gpsimd.dma_start(gt[:], gs[:])
        nc.gpsimd.dma_start(xsT[:], xst[:])
        nc.gpsimd.dma_start(bsb[:], bia[:])
        for g in S_SINGLES:
            nc.gpsimd.dma_start(whS[g][:], wv8p[:, POS[g] * GW:(POS[g] + 1) * GW])

        # sync/HWDGE ring: x codes, A-pair fp8 slabs, B-pair bf16 slabs.
        nc.sync.dma_start(xva[:], xvp[:])
        for a, b in A_PAIRS:
            nc.sync.dma_start(w8[a][:],
                              wv8p[:, POS[a] * GW:(POS[a] + 2) * GW])
        nc.sync.dma_start(ptB[9][:], wvbp[:, 0:2 * GW])
        nc.sync.dma_start(ptB[13][:], wvbp[:, 2 * GW:4 * GW])

        psum = ppool.tile([M, NS], dt.float32)
        ones = const.tile([1, M], dt.bfloat16)
        gcol = const.tile([P, 1], dt.float32)
        xsb = const.tile([P, CB * M], dt.bfloat16)
        xhat = const.tile([P, CHUNKS * M], dt.bfloat16)

        def emit_mms(g, c, wh, col0, stop=False):
            t = g * CB + c
            for h in range(2):
                nc.tensor.matmul(
                    psum[:, h * 512:(h + 1) * 512],
                    xhat[:, t * M:(t + 1) * M],
                    wh[:, col0 + c * NS + h * 512: col0 + c * NS + (h + 1) * 512],
                    start=False,
                    stop=stop,
                )

        wsT_b2 = wsT[:].unsqueeze(1).broadcast_to([P, 2, GW])

        def pair_tt(tle):
            nc.vector.tensor_mul(
                tle[:].rearrange("p (q w) -> p q w", q=2),
                tle[:].rearrange("p (q w) -> p q w", q=2),
                wsT_b2,
            )

        # ---- g0 chunk TTs (head) ----
        for c in range(CB):
            cs = slice(c * NQ, (c + 1) * NQ)
            nc.vector.tensor_mul(whS0[:, cs], whS0[:, cs], wsT[:, cs])

        # ---- x-side ----
        nc.vector.tensor_mul(gcol[:], gt[:, 0:1], gt[:, 1:2])
        nc.vector.tensor_scalar_mul(xsb[:], xsT[:], gcol[:])
        xsb_b = xsb[:].unsqueeze(1).broadcast_to([P, J, CB * M])
        nc.vector.tensor_mul(
            xhat[:].rearrange("p (j w) -> p j w", j=J),
            xva[:].rearrange("p (j w) -> p j w", j=J),
            xsb_b,
        )
        nc.vector.memset(ones[:], 1.0)

        # bias seeds PSUM (start=True clears)
        for h in range(2):
            nc.tensor.matmul(
                psum[:, h * 512:(h + 1) * 512],
                ones[:1, :],
                bsb[:1, h * 512:(h + 1) * 512],
                start=True,
                stop=False,
            )
        for c in range(CB):
            emit_mms(0, c, whS0, 0)

        # ---- main line ----
        # g1, g3: S singles
        for g in (1, 3):
            nc.vector.tensor_mul(whS[g][:], whS[g][:], wsT[:])
            for c in range(CB):
                emit_mms(g, c, whS[g], 0)
        # A-pair (2,4)
        nc.scalar.copy(ptA[2][:], w8[2][:])
        pair_tt(ptA[2])
        for c in range(CB):
            emit_mms(2, c, ptA[2], 0)
        for c in range(CB):
            emit_mms(4, c, ptA[2], GW)
        # g5, g7
        for g in (5, 7):
            nc.vector.tensor_mul(whS[g][:], whS[g][:], wsT[:])
            for c in range(CB):
                emit_mms(g, c, whS[g], 0)
        # B-pair (9,11)
        pair_tt(ptB[9])
        for c in range(CB):
            emit_mms(9, c, ptB[9], 0)
        for c in range(CB):
            emit_mms(11, c, ptB[9], GW)
        # B-pair (10,12)
        pair_tt(ptB[10])
        for c in range(CB):
            emit_mms(10, c, ptB[10], 0)
        for c in range(CB):
            emit_mms(12, c, ptB[10], GW)
        # A-pair (6,8)
        nc.scalar.copy(ptA[6][:], w8[6][:])
        pair_tt(ptA[6])
        for c in range(CB):
            emit_mms(6, c, ptA[6], 0)
        for c in range(CB):
            emit_mms(8, c, ptA[6], GW)
        # B-pair (13,15)
        pair_tt(ptB[13])
        for c in range(CB):
            emit_mms(13, c, ptB[13], 0)
        for c in range(CB):
            emit_mms(15, c, ptB[13], GW)
        # g14: ACT chunk-casts + chunk TTs (tail)
        for c in range(CB):
            cs = slice(c * NQ, (c + 1) * NQ)
            nc.scalar.copy(whA14[:, cs], w8[14][:, cs])
            nc.vector.tensor_mul(whA14[:, cs], whA14[:, cs], wsT[:, cs])
            emit_mms(14, c, whA14, 0, stop=(c == CB - 1))

        # ---- tail ----
        osb = const.tile([M, NS], dt.bfloat16)
        nc.vector.tensor_copy(osb[:, 0:512], psum[:, 0:512])
        nc.scalar.copy(osb[:, 512:NS], psum[:, 512:NS])
        nc.scalar.dma_start(out[:], osb[:])

    return nc


def _perm_k(vals_2d: np.ndarray) -> np.ndarray:
    """[R, K] fp values -> [K, R] with K permuted as r = j*B + b."""
    r = vals_2d.shape[0]
    return (
        vals_2d.reshape(r, B, BLOCK).transpose(2, 1, 0).reshape(K, r)
    )


def _swz(rows_2d: np.ndarray, width: int) -> np.ndarray:
    """[n_chunks*128, width] -> [128, n_chunks*width]: row p holds chunk-major
    data for partition p (per-partition-contiguous DMA layout)."""
    n = rows_2d.shape[0] // P
    return np.ascontiguousarray(
        rows_2d.reshape(n, P, width).transpose(1, 0, 2).reshape(P, n * width)
    )


B_POS = (9, 10, 11, 12, 14, 15)     # layout positions shipped as bf16
F_POS = (0, 1, 2, 3, 4, 5, 6, 7, 8, 13)   # positions shipped as fp8


def prepare_in_maps(**inputs) -> list[dict[str, np.ndarray]]:
    x = np.asarray(inputs["x"]).astype(np.int64)
    xs = np.asarray(inputs["x_scale"], dtype=np.float32)
    gx = np.float32(np.asarray(inputs["x_global_scale"]).reshape(-1)[0])
    w = np.asarray(inputs["weight"]).astype(np.int64)
    ws = np.asarray(inputs["weight_scale"], dtype=np.float32)
    gw = np.float32(np.asarray(inputs["weight_global_scale"]).reshape(-1)[0])
    b = np.asarray(inputs["bias"], dtype=np.float32)

    FP8 = ml_dtypes.float8_e4m3
    xvp = _swz(_perm_k(FP4_LUT[x]).astype(BF16), M)                  # [128, 4096]
    xst = _swz(np.ascontiguousarray(xs.T), M).astype(BF16)           # [128, 256]
    gs = np.tile(np.array([[gx, gw]], dtype=np.float32), (P, 1))

    wv = FP4_LUT[w]                                                  # [N, K] f32
    in_maps = []
    for c in range(NCORES):
        sl = slice(c * NS, (c + 1) * NS)
        wvp = _swz(_perm_k(wv[sl]), NS)                              # [128, 64*NS] f32
        wg = wvp.reshape(P, J, GW)[:, list(NEWORD), :]               # role order
        wv8p = np.ascontiguousarray(
            wg[:, list(F_POS), :].reshape(P, len(F_POS) * GW)).astype(FP8)
        wvbp = np.ascontiguousarray(
            wg[:, list(B_POS), :].reshape(P, len(B_POS) * GW)).astype(BF16)
        in_maps.append({
            "wv8p": wv8p,
            "wvbp": wvbp,
            "wst": _swz(ws[sl].T.astype(BF16), NS),                  # [128, 4*NS]
            "xvp": xvp,
            "xst": xst,
            "gs": gs,
            "bia": np.ascontiguousarray(b[sl].reshape(1, NS)).astype(BF16),
        })
    return in_maps


LAST_RESULTS = None


def kernel(**inputs) -> np.ndarray:
    global LAST_RESULTS
    if "nc" not in _CACHE:
        _CACHE["nc"] = _build_program()
    nc = _CACHE["nc"]

    in_maps = prepare_in_maps(**inputs)
    res = run_bass_kernel_spmd(nc, in_maps, core_ids=list(range(NCORES)))
    LAST_RESULTS = res
    out = np.concatenate([res.results[c]["out"] for c in range(NCORES)], axis=1)
    return out.astype(BF16)


# revision 10
# speedup vs baseline: 1.4378x; 1.4378x over previous
"""NVFP4 (E2M1, block-16) dequant matmul on 8 TRN2 NeuronCores — v6.

out[m, n] = sum_k (LUT[x[m,k]] * xs[m,k//16] * gx) * (LUT[w[n,k]] * ws[n,k//16] * gw) + bias[n]

Sharding: tensor-parallel along N: each of the 8 cores owns 1024 output
columns (weight/weight_scale/bias rows); x replicated.

v6 design (measured-fact driven):
  - Delivery: ONE SWDGE ring carries every load in exact consumption
    order (multiple rings round-robin at packet granularity and dilute
    critical early transfers behind bulk; a single FIFO ring measured
    ~390 GB/s of SBUF-write in v1). All tiles are dedicated (full
    prefetch run-ahead, no pool-slot throttling).
  - 8 groups land as bf16 via cast-DMA (write 1.05MB each); 8 land as
    fp8 slabs (write 0.52MB) and ScalarE pair-casts them — its 2x
    expansion happens off-fabric, balancing fabric (~15MB) vs the ACT
    chain (~28us) vs DVE (~39us).
  - DVE runs every dequant multiply at 2x bf16, pair-fused [128,2,4096]
    against a stride-0-broadcast wsT; x dequant is one broadcast TT.
    GpSimd runs zero tensor ops (shared SBUF port would halve both).
  - g0 chunk-granular at the head (halved DMAs + split wsT), g15
    chunk-granular at the tail (its data lands mid-stream, no cast).

Host-side marshaling stays format-only (LUT decode + layout + dtype cast);
all reference arithmetic (scale multiplies, matmul, bias) is on device.
"""

import json
from contextlib import ExitStack

import ml_dtypes
import numpy as np

import concourse.bass as bass
import concourse.mybir as mybir
import concourse.tile as tile
from concourse.bass_utils import run_bass_kernel_spmd


def _split_multi_waits(m: dict) -> dict:
    """This walrus build allows at most one sync-wait command per instruction.
    Hoist extra waits into standalone EventSemaphore instructions issued just
    before the owning instruction on the same engine queue (semantically
    identical: the engine stalls in order)."""
    for fn in m["functions"]:
        for blk in fn["blocks"]:
            new = []
            ctr = 0
            for inst in blk["instructions"]:
                si = inst.get("sync_info")
                waits = (si or {}).get("on_wait") or []
                if len(waits) > 1:
                    for w in waits[:-1]:
                        new.append({
                            "debug": inst.get("debug", 0),
                            "engine": inst["engine"],
                            "ins": [],
                            "outs": [],
                            "name": f"{inst['name']}-hw{ctr}",
                            "opcode": "EventSemaphore",
                            "sync_info": {"on_update": [], "on_wait": [w]},
                        })
                        ctr += 1
                    si["on_wait"] = [waits[-1]]
                new.append(inst)
            blk["instructions"] = new
    return m


class _SplitWaitBass(bass.Bass):
    def to_json_bytes(self) -> bytes:
        m = json.loads(super().to_json_bytes())
        return json.dumps(_split_multi_waits(m)).encode()


BF16 = ml_dtypes.bfloat16
FP4_LUT = np.array(
    [0.0, 0.5, 1.0, 1.5, 2.0, 3.0, 4.0, 6.0,
     -0.0, -0.5, -1.0, -1.5, -2.0, -3.0, -4.0, -6.0],
    dtype=np.float32,
)

M, K, N = 64, 8192, 8192
NCORES = 8
NS = N // NCORES        # 1024 output columns per core
BLOCK = 16
B = K // BLOCK          # 512 scale blocks along K
P = 128                 # partitions
CHUNKS = K // P         # 64 K-chunks
CB = B // P             # 4 scale-chunk columns (c index)
J = BLOCK               # 16 j-groups (one group = CB chunks = 512 rows)
GW = CB * NS            # 4096 columns per weight group tile
NQ = NS                 # 1024 columns per chunk of a group tile

# Host wvp layout positions (group order in DRAM):
#   [0 | 1,3 | 2,4 | 5,7 | 6,8 | 9,11 | 10,12 | 13,14 | 15]
# S (cast-DMA bf16): 0, (1,3), (5,7), (9,11), 15; A (fp8+ACT cast):
# (2,4), (6,8), (10,12), (13,14).
NEWORD = (0, 1, 3, 2, 4, 5, 7, 6, 8, 9, 11, 10, 12, 13, 14, 15)
POS = {g: i for i, g in enumerate(NEWORD)}
S_PAIRS = ((1, 3), (5, 7), (9, 11))
A_PAIRS = ((2, 4), (6, 8), (10, 12), (13, 14))

_CACHE: dict = {}


def _build_program() -> bass.Bass:
    nc = _SplitWaitBass("TRN2", target_bir_lowering=False, debug=False,
                        num_devices=NCORES)
    dt = mybir.dt

    wvp = nc.dram_tensor("wvp", [P, CHUNKS * NS], dt.float8e4,
                         kind="ExternalInput").ap()
    wst = nc.dram_tensor("wst", [P, GW], dt.bfloat16,
                         kind="ExternalInput").ap()
    xvp = nc.dram_tensor("xvp", [P, CHUNKS * M], dt.bfloat16,
                         kind="ExternalInput").ap()
    xst = nc.dram_tensor("xst", [P, CB * M], dt.bfloat16,
                         kind="ExternalInput").ap()
    gs = nc.dram_tensor("gs", [P, 2], dt.float32, kind="ExternalInput").ap()
    bia = nc.dram_tensor("bia", [1, NS], dt.bfloat16, kind="ExternalInput").ap()
    out = nc.dram_tensor("out", [M, NS], dt.bfloat16, kind="ExternalOutput").ap()

    def slab(g, n=1):
        return wvp[:, POS[g] * GW:(POS[g] + n) * GW]

    with tile.TileContext(nc) as tc, ExitStack() as ctx:
        const = ctx.enter_context(tc.tile_pool(name="const", bufs=1))
        w8pool = ctx.enter_context(tc.tile_pool(name="w8", bufs=1))
        whpool = ctx.enter_context(tc.tile_pool(name="wh", bufs=1))
        ppool = ctx.enter_context(tc.tile_pool(name="acc", bufs=1, space="PSUM"))

        wsT = const.tile([P, GW], dt.bfloat16)
        xva = const.tile([P, CHUNKS * M], dt.bfloat16)
        gt = const.tile([P, 2], dt.float32)
        xsT = const.tile([P, CB * M], dt.bfloat16)
        bsb = const.tile([1, NS], dt.bfloat16)
        whS0 = whpool.tile([P, GW], dt.bfloat16, name="whS0")
        whS15 = whpool.tile([P, GW], dt.bfloat16, name="whS15")
        ptS: dict = {}
        for a, b in S_PAIRS:
            ptS[a] = whpool.tile([P, 2 * GW], dt.bfloat16, name=f"ptS{a}")
        ptA: dict = {}
        w8: dict = {}
        for a, b in A_PAIRS:
            ptA[a] = whpool.tile([P, 2 * GW], dt.bfloat16, name=f"ptA{a}")
            w8[a] = w8pool.tile([P, 2 * GW], dt.float8e4, name=f"w8_{a}")

        # ---- single SWDGE stream, consumption order ----
        H = 2 * NQ
        nc.gpsimd.dma_start(wsT[:, 0:H], wst[:, 0:H])            # wsT half 0
        nc.gpsimd.dma_start(whS0[:, 0:H], wvp[:, 0:H])           # g0 half 0
        nc.gpsimd.dma_start(whS0[:, H:GW], wvp[:, H:GW])         # g0 half 1
        nc.gpsimd.dma_start(wsT[:, H:GW], wst[:, H:GW])          # wsT half 1
        nc.gpsimd.dma_start(gt[:], gs[:])
        nc.gpsimd.dma_start(xsT[:], xst[:])
        nc.gpsimd.dma_start(w8[2][:], slab(2, 2))                # A slab (2,4)
        nc.gpsimd.dma_start(xva[:], xvp[:])
        nc.gpsimd.dma_start(bsb[:], bia[:])
        nc.gpsimd.dma_start(w8[6][:], slab(6, 2))                # A slab (6,8)
        nc.gpsimd.dma_start(ptS[1][:], slab(1, 2))               # S pair (1,3)
        nc.gpsimd.dma_start(w8[10][:], slab(10, 2))              # A slab (10,12)
        nc.gpsimd.dma_start(ptS[5][:], slab(5, 2))               # S pair (5,7)
        nc.gpsimd.dma_start(w8[13][:], slab(13, 2))              # A slab (13,14)
        nc.gpsimd.dma_start(ptS[9][:], slab(9, 2))               # S pair (9,11)
        nc.gpsimd.dma_start(whS15[:], slab(15, 1))               # g15

        psum = ppool.tile([M, NS], dt.float32)
        ones = const.tile([1, M], dt.bfloat16)
        gcol = const.tile([P, 1], dt.float32)
        xsb = const.tile([P, CB * M], dt.bfloat16)
        xhat = const.tile([P, CHUNKS * M], dt.bfloat16)

        def emit_mms(g, c, wh, col0, stop=False):
            t = g * CB + c
            for h in range(2):
                nc.tensor.matmul(
                    psum[:, h * 512:(h + 1) * 512],
                    xhat[:, t * M:(t + 1) * M],
                    wh[:, col0 + c * NS + h * 512: col0 + c * NS + (h + 1) * 512],
                    start=False,
                    stop=stop,
                )

        wsT_b2 = wsT[:].unsqueeze(1).broadcast_to([P, 2, GW])

        def pair_tt(tle):
            nc.vector.tensor_mul(
                tle[:].rearrange("p (q w) -> p q w", q=2),
                tle[:].rearrange("p (q w) -> p q w", q=2),
                wsT_b2,
            )

        # ---- g0 chunk TTs (head; c0/c1 gated on half 0) ----
        for c in range(CB):
            cs = slice(c * NQ, (c + 1) * NQ)
            nc.vector.tensor_mul(whS0[:, cs], whS0[:, cs], wsT[:, cs])

        # ---- x-side ----
        nc.vector.tensor_mul(gcol[:], gt[:, 0:1], gt[:, 1:2])
        nc.vector.tensor_scalar_mul(xsb[:], xsT[:], gcol[:])
        xsb_b = xsb[:].unsqueeze(1).broadcast_to([P, J, CB * M])
        nc.vector.tensor_mul(
            xhat[:].rearrange("p (j w) -> p j w", j=J),
            xva[:].rearrange("p (j w) -> p j w", j=J),
            xsb_b,
        )
        nc.vector.memset(ones[:], 1.0)

        for h in range(2):
            nc.tensor.matmul(
                psum[:, h * 512:(h + 1) * 512],
                ones[:1, :],
                bsb[:1, h * 512:(h + 1) * 512],
                start=True,
                stop=False,
            )
        for c in range(CB):
            emit_mms(0, c, whS0, 0)

        # ---- main line: A/S pairs interleaved by arrival ----
        def do_pair(a, b, tle, is_a):
            if is_a:
                nc.scalar.copy(tle[:], w8[a][:])     # fused 2-group cast
            pair_tt(tle)
            for c in range(CB):
                emit_mms(a, c, tle, 0)
            for c in range(CB):
                emit_mms(b, c, tle, GW)

        do_pair(2, 4, ptA[2], True)
        do_pair(1, 3, ptS[1], False)
        do_pair(6, 8, ptA[6], True)
        do_pair(5, 7, ptS[5], False)
        do_pair(10, 12, ptA[10], True)
        do_pair(9, 11, ptS[9], False)
        do_pair(13, 14, ptA[13], True)

        # ---- g15 chunk TTs (tail; data landed mid-stream, no cast) ----
        for c in range(CB):
            cs = slice(c * NQ, (c + 1) * NQ)
            nc.vector.tensor_mul(whS15[:, cs], whS15[:, cs], wsT[:, cs])
            emit_mms(15, c, whS15, 0, stop=(c == CB - 1))

        # ---- tail ----
        osb = const.tile([M, NS], dt.bfloat16)
        nc.vector.tensor_copy(osb[:, 0:512], psum[:, 0:512])
        nc.scalar.copy(osb[:, 512:NS], psum[:, 512:NS])
        nc.scalar.dma_start(out[:], osb[:])

    return nc


def _perm_k(vals_2d: np.ndarray) -> np.ndarray:
    """[R, K] fp values -> [K, R] with K permuted as r = j*B + b."""
    r = vals_2d.shape[0]
    return (
        vals_2d.reshape(r, B, BLOCK).transpose(2, 1, 0).reshape(K, r)
    )


def _swz(rows_2d: np.ndarray, width: int) -> np.ndarray:
    """[n_chunks*128, width] -> [128, n_chunks*width]: row p holds chunk-major
    data for partition p (per-partition-contiguous DMA layout)."""
    n = rows_2d.shape[0] // P
    return np.ascontiguousarray(
        rows_2d.reshape(n, P, width).transpose(1, 0, 2).reshape(P, n * width)
    )


def prepare_in_maps(**inputs) -> list[dict[str, np.ndarray]]:
    x = np.asarray(inputs["x"]).astype(np.int64)
    xs = np.asarray(inputs["x_scale"], dtype=np.float32)
    gx = np.float32(np.asarray(inputs["x_global_scale"]).reshape(-1)[0])
    w = np.asarray(inputs["weight"]).astype(np.int64)
    ws = np.asarray(inputs["weight_scale"], dtype=np.float32)
    gw = np.float32(np.asarray(inputs["weight_global_scale"]).reshape(-1)[0])
    b = np.asarray(inputs["bias"], dtype=np.float32)

    FP8 = ml_dtypes.float8_e4m3
    xvp = _swz(_perm_k(FP4_LUT[x]).astype(BF16), M)                  # [128, 4096]
    xst = _swz(np.ascontiguousarray(xs.T), M).astype(BF16)           # [128, 256]
    gs = np.tile(np.array([[gx, gw]], dtype=np.float32), (P, 1))

    wv = FP4_LUT[w]                                                  # [N, K] f32
    in_maps = []
    for c in range(NCORES):
        sl = slice(c * NS, (c + 1) * NS)
        wvp = _swz(_perm_k(wv[sl]).astype(FP8), NS)                  # [128, 64*NS]
        wg = wvp.reshape(P, J, GW)[:, list(NEWORD), :]
        in_maps.append({
            "wvp": np.ascontiguousarray(wg.reshape(P, J * GW)),
            "wst": _swz(ws[sl].T.astype(BF16), NS),                  # [128, 4*NS]
            "xvp": xvp,
            "xst": xst,
            "gs": gs,
            "bia": np.ascontiguousarray(b[sl].reshape(1, NS)).astype(BF16),
        })
    return in_maps


LAST_RESULTS = None


def kernel(**inputs) -> np.ndarray:
    global LAST_RESULTS
    if "nc" not in _CACHE:
        _CACHE["nc"] = _build_program()
    nc = _CACHE["nc"]

    in_maps = prepare_in_maps(**inputs)
    res = run_bass_kernel_spmd(nc, in_maps, core_ids=list(range(NCORES)))
    LAST_RESULTS = res
    out = np.concatenate([res.results[c]["out"] for c in range(NCORES)], axis=1)
    return out.astype(BF16)


# revision 11
# speedup vs baseline: 1.4473x; 1.0066x over previous
"""NVFP4 (E2M1, block-16) dequant matmul on 8 TRN2 NeuronCores — v7.

out[m, n] = sum_k (LUT[x[m,k]] * xs[m,k//16] * gx) * (LUT[w[n,k]] * ws[n,k//16] * gw) + bias[n]

Sharding: tensor-parallel along N: each of the 8 cores owns 1024 output
columns (weight/weight_scale/bias rows); x replicated.

v6 design (measured-fact driven):
  - Delivery: ONE SWDGE ring carries every load in exact consumption
    order (multiple rings round-robin at packet granularity and dilute
    critical early transfers behind bulk; a single FIFO ring measured
    ~390 GB/s of SBUF-write in v1). All tiles are dedicated (full
    prefetch run-ahead, no pool-slot throttling).
  - 8 groups land as bf16 via cast-DMA (write 1.05MB each); 8 land as
    fp8 slabs (write 0.52MB) and ScalarE pair-casts them — its 2x
    expansion happens off-fabric, balancing fabric (~15MB) vs the ACT
    chain (~28us) vs DVE (~39us).
  - DVE runs every dequant multiply at 2x bf16, pair-fused [128,2,4096]
    against a stride-0-broadcast wsT; x dequant is one broadcast TT.
    GpSimd runs zero tensor ops (shared SBUF port would halve both).
  - g0 chunk-granular at the head (halved DMAs + split wsT), g15
    chunk-granular at the tail (its data lands mid-stream, no cast).

Host-side marshaling stays format-only (LUT decode + layout + dtype cast);
all reference arithmetic (scale multiplies, matmul, bias) is on device.
"""

import json
from contextlib import ExitStack

import ml_dtypes
import numpy as np

import concourse.bass as bass
import concourse.mybir as mybir
import concourse.tile as tile
from concourse.bass_utils import run_bass_kernel_spmd


def _split_multi_waits(m: dict) -> dict:
    """This walrus build allows at most one sync-wait command per instruction.
    Hoist extra waits into standalone EventSemaphore instructions issued just
    before the owning instruction on the same engine queue (semantically
    identical: the engine stalls in order)."""
    for fn in m["functions"]:
        for blk in fn["blocks"]:
            new = []
            ctr = 0
            for inst in blk["instructions"]:
                si = inst.get("sync_info")
                waits = (si or {}).get("on_wait") or []
                if len(waits) > 1:
                    for w in waits[:-1]:
                        new.append({
                            "debug": inst.get("debug", 0),
                            "engine": inst["engine"],
                            "ins": [],
                            "outs": [],
                            "name": f"{inst['name']}-hw{ctr}",
                            "opcode": "EventSemaphore",
                            "sync_info": {"on_update": [], "on_wait": [w]},
                        })
                        ctr += 1
                    si["on_wait"] = [waits[-1]]
                new.append(inst)
            blk["instructions"] = new
    return m


class _SplitWaitBass(bass.Bass):
    def to_json_bytes(self) -> bytes:
        m = json.loads(super().to_json_bytes())
        return json.dumps(_split_multi_waits(m)).encode()


BF16 = ml_dtypes.bfloat16
FP4_LUT = np.array(
    [0.0, 0.5, 1.0, 1.5, 2.0, 3.0, 4.0, 6.0,
     -0.0, -0.5, -1.0, -1.5, -2.0, -3.0, -4.0, -6.0],
    dtype=np.float32,
)

M, K, N = 64, 8192, 8192
NCORES = 8
NS = N // NCORES        # 1024 output columns per core
BLOCK = 16
B = K // BLOCK          # 512 scale blocks along K
P = 128                 # partitions
CHUNKS = K // P         # 64 K-chunks
CB = B // P             # 4 scale-chunk columns (c index)
J = BLOCK               # 16 j-groups (one group = CB chunks = 512 rows)
GW = CB * NS            # 4096 columns per weight group tile
NQ = NS                 # 1024 columns per chunk of a group tile

# Host wvp layout positions (group order in DRAM):
#   [0 | 1,3 | 2,4 | 5,7 | 6,8 | 9,11 | 10,12 | 13,14 | 15]
# S (cast-DMA bf16): 0, (1,3), (5,7), (9,11), 15; A (fp8+ACT cast):
# (2,4), (6,8), (10,12), (13,14).
NEWORD = (0, 1, 3, 2, 4, 5, 7, 6, 8, 9, 11, 10, 12, 13, 14, 15)
POS = {g: i for i, g in enumerate(NEWORD)}
S_PAIRS = ((1, 3), (5, 7), (9, 11))
A_PAIRS = ((2, 4), (6, 8), (10, 12), (13, 14))

_CACHE: dict = {}


def _build_program() -> bass.Bass:
    nc = _SplitWaitBass("TRN2", target_bir_lowering=False, debug=False,
                        num_devices=NCORES)
    dt = mybir.dt

    wvp = nc.dram_tensor("wvp", [P, CHUNKS * NS], dt.float8e4,
                         kind="ExternalInput").ap()
    wst = nc.dram_tensor("wst", [P, GW], dt.bfloat16,
                         kind="ExternalInput").ap()
    xvp = nc.dram_tensor("xvp", [P, CHUNKS * M], dt.bfloat16,
                         kind="ExternalInput").ap()
    xst = nc.dram_tensor("xst", [P, CB * M], dt.bfloat16,
                         kind="ExternalInput").ap()
    gs = nc.dram_tensor("gs", [P, 2], dt.float32, kind="ExternalInput").ap()
    bia = nc.dram_tensor("bia", [1, NS], dt.bfloat16, kind="ExternalInput").ap()
    out = nc.dram_tensor("out", [M, NS], dt.bfloat16, kind="ExternalOutput").ap()

    def slab(g, n=1):
        return wvp[:, POS[g] * GW:(POS[g] + n) * GW]

    with tile.TileContext(nc) as tc, ExitStack() as ctx:
        const = ctx.enter_context(tc.tile_pool(name="const", bufs=1))
        w8pool = ctx.enter_context(tc.tile_pool(name="w8", bufs=1))
        whpool = ctx.enter_context(tc.tile_pool(name="wh", bufs=1))
        ppool = ctx.enter_context(tc.tile_pool(name="acc", bufs=1, space="PSUM"))

        wsT = const.tile([P, GW], dt.bfloat16)
        xva = const.tile([P, CHUNKS * M], dt.bfloat16)
        gt = const.tile([P, 2], dt.float32)
        xsT = const.tile([P, CB * M], dt.bfloat16)
        bsb = const.tile([1, NS], dt.bfloat16)
        whS0 = whpool.tile([P, GW], dt.bfloat16, name="whS0")
        whS15 = whpool.tile([P, GW], dt.bfloat16, name="whS15")
        ptS: dict = {}
        for a, b in S_PAIRS:
            ptS[a] = whpool.tile([P, 2 * GW], dt.bfloat16, name=f"ptS{a}")
        ptA: dict = {}
        w8: dict = {}
        for a, b in A_PAIRS:
            ptA[a] = whpool.tile([P, 2 * GW], dt.bfloat16, name=f"ptA{a}")
            w8[a] = w8pool.tile([P, 2 * GW], dt.float8e4, name=f"w8_{a}")

        # ---- single SWDGE stream, consumption order ----
        H = 2 * NQ
        nc.gpsimd.dma_start(wsT[:, 0:H], wst[:, 0:H])            # wsT half 0
        nc.gpsimd.dma_start(whS0[:, 0:H], wvp[:, 0:H])           # g0 half 0
        nc.gpsimd.dma_start(whS0[:, H:GW], wvp[:, H:GW])         # g0 half 1
        nc.gpsimd.dma_start(w8[2][:], slab(2, 2))                # A slab (2,4)
        nc.gpsimd.dma_start(wsT[:, H:GW], wst[:, H:GW])          # wsT half 1
        nc.gpsimd.dma_start(gt[:], gs[:])
        nc.gpsimd.dma_start(xsT[:], xst[:])
        nc.gpsimd.dma_start(xva[:], xvp[:])
        nc.gpsimd.dma_start(bsb[:], bia[:])
        nc.gpsimd.dma_start(w8[6][:], slab(6, 2))                # A slab (6,8)
        nc.gpsimd.dma_start(ptS[1][:], slab(1, 2))               # S pair (1,3)
        nc.gpsimd.dma_start(w8[10][:], slab(10, 2))              # A slab (10,12)
        nc.gpsimd.dma_start(ptS[5][:], slab(5, 2))               # S pair (5,7)
        nc.gpsimd.dma_start(w8[13][:], slab(13, 2))              # A slab (13,14)
        nc.gpsimd.dma_start(ptS[9][:], slab(9, 2))               # S pair (9,11)
        nc.gpsimd.dma_start(whS15[:], slab(15, 1))               # g15

        psum = ppool.tile([M, NS], dt.float32)
        ones = const.tile([1, M], dt.bfloat16)
        gcol = const.tile([P, 1], dt.float32)
        xsb = const.tile([P, CB * M], dt.bfloat16)
        xhat = const.tile([P, CHUNKS * M], dt.bfloat16)

        def emit_mms(g, c, wh, col0, stop=False):
            t = g * CB + c
            for h in range(2):
                nc.tensor.matmul(
                    psum[:, h * 512:(h + 1) * 512],
                    xhat[:, t * M:(t + 1) * M],
                    wh[:, col0 + c * NS + h * 512: col0 + c * NS + (h + 1) * 512],
                    start=False,
                    stop=stop,
                )

        wsT_b2 = wsT[:].unsqueeze(1).broadcast_to([P, 2, GW])

        def pair_tt(tle):
            nc.vector.tensor_mul(
                tle[:].rearrange("p (q w) -> p q w", q=2),
                tle[:].rearrange("p (q w) -> p q w", q=2),
                wsT_b2,
            )

        # ---- g0 chunk TTs (head; c0/c1 gated on half 0) ----
        for c in range(CB):
            cs = slice(c * NQ, (c + 1) * NQ)
            nc.vector.tensor_mul(whS0[:, cs], whS0[:, cs], wsT[:, cs])

        # ---- x-side ----
        nc.vector.tensor_mul(gcol[:], gt[:, 0:1], gt[:, 1:2])
        nc.vector.tensor_scalar_mul(xsb[:], xsT[:], gcol[:])
        xsb_b = xsb[:].unsqueeze(1).broadcast_to([P, J, CB * M])
        nc.vector.tensor_mul(
            xhat[:].rearrange("p (j w) -> p j w", j=J),
            xva[:].rearrange("p (j w) -> p j w", j=J),
            xsb_b,
        )
        nc.vector.memset(ones[:], 1.0)

        for h in range(2):
            nc.tensor.matmul(
                psum[:, h * 512:(h + 1) * 512],
                ones[:1, :],
                bsb[:1, h * 512:(h + 1) * 512],
                start=True,
                stop=False,
            )
        for c in range(CB):
            emit_mms(0, c, whS0, 0)

        # ---- main line: A/S pairs interleaved by arrival ----
        def do_pair(a, b, tle, is_a):
            if is_a:
                nc.scalar.copy(tle[:], w8[a][:])     # fused 2-group cast
            pair_tt(tle)
            for c in range(CB):
                emit_mms(a, c, tle, 0)
            for c in range(CB):
                emit_mms(b, c, tle, GW)

        do_pair(2, 4, ptA[2], True)
        do_pair(1, 3, ptS[1], False)
        do_pair(6, 8, ptA[6], True)
        do_pair(5, 7, ptS[5], False)
        do_pair(10, 12, ptA[10], True)
        do_pair(9, 11, ptS[9], False)
        do_pair(13, 14, ptA[13], True)

        # ---- g15 chunk TTs (tail; data landed mid-stream, no cast) ----
        for c in range(CB):
            cs = slice(c * NQ, (c + 1) * NQ)
            nc.vector.tensor_mul(whS15[:, cs], whS15[:, cs], wsT[:, cs])
            emit_mms(15, c, whS15, 0, stop=(c == CB - 1))

        # ---- tail ----
        osb = const.tile([M, NS], dt.bfloat16)
        nc.vector.tensor_copy(osb[:, 0:512], psum[:, 0:512])
        nc.scalar.copy(osb[:, 512:NS], psum[:, 512:NS])
        nc.sync.dma_start(out[:, 0:512], osb[:, 0:512])
        nc.scalar.dma_start(out[:, 512:NS], osb[:, 512:NS])

    return nc


def _perm_k(vals_2d: np.ndarray) -> np.ndarray:
    """[R, K] fp values -> [K, R] with K permuted as r = j*B + b."""
    r = vals_2d.shape[0]
    return (
        vals_2d.reshape(r, B, BLOCK).transpose(2, 1, 0).reshape(K, r)
    )


def _swz(rows_2d: np.ndarray, width: int) -> np.ndarray:
    """[n_chunks*128, width] -> [128, n_chunks*width]: row p holds chunk-major
    data for partition p (per-partition-contiguous DMA layout)."""
    n = rows_2d.shape[0] // P
    return np.ascontiguousarray(
        rows_2d.reshape(n, P, width).transpose(1, 0, 2).reshape(P, n * width)
    )


def prepare_in_maps(**inputs) -> list[dict[str, np.ndarray]]:
    x = np.asarray(inputs["x"]).astype(np.int64)
    xs = np.asarray(inputs["x_scale"], dtype=np.float32)
    gx = np.float32(np.asarray(inputs["x_global_scale"]).reshape(-1)[0])
    w = np.asarray(inputs["weight"]).astype(np.int64)
    ws = np.asarray(inputs["weight_scale"], dtype=np.float32)
    gw = np.float32(np.asarray(inputs["weight_global_scale"]).reshape(-1)[0])
    b = np.asarray(inputs["bias"], dtype=np.float32)

    FP8 = ml_dtypes.float8_e4m3
    xvp = _swz(_perm_k(FP4_LUT[x]).astype(BF16), M)                  # [128, 4096]
    xst = _swz(np.ascontiguousarray(xs.T), M).astype(BF16)           # [128, 256]
    gs = np.tile(np.array([[gx, gw]], dtype=np.float32), (P, 1))

    wv = FP4_LUT[w]                                                  # [N, K] f32
    in_maps = []
    for c in range(NCORES):
        sl = slice(c * NS, (c + 1) * NS)
        wvp = _swz(_perm_k(wv[sl]).astype(FP8), NS)                  # [128, 64*NS]
        wg = wvp.reshape(P, J, GW)[:, list(NEWORD), :]
        in_maps.append({
            "wvp": np.ascontiguousarray(wg.reshape(P, J * GW)),
            "wst": _swz(ws[sl].T.astype(BF16), NS),                  # [128, 4*NS]
            "xvp": xvp,
            "xst": xst,
            "gs": gs,
            "bia": np.ascontiguousarray(b[sl].reshape(1, NS)).astype(BF16),
        })
    return in_maps


LAST_RESULTS = None


def kernel(**inputs) -> np.ndarray:
    global LAST_RESULTS
    if "nc" not in _CACHE:
        _CACHE["nc"] = _build_program()
    nc = _CACHE["nc"]

    in_maps = prepare_in_maps(**inputs)
    res = run_bass_kernel_spmd(nc, in_maps, core_ids=list(range(NCORES)))
    LAST_RESULTS = res
    out = np.concatenate([res.results[c]["out"] for c in range(NCORES)], axis=1)
    return out.astype(BF16)


# revision 13
# speedup vs baseline: 1.4664x; 1.0132x over previous
"""NVFP4 (E2M1, block-16) dequant matmul on 8 TRN2 NeuronCores — v7.

out[m, n] = sum_k (LUT[x[m,k]] * xs[m,k//16] * gx) * (LUT[w[n,k]] * ws[n,k//16] * gw) + bias[n]

Sharding: tensor-parallel along N: each of the 8 cores owns 1024 output
columns (weight/weight_scale/bias rows); x replicated.

v6 design (measured-fact driven):
  - Delivery: ONE SWDGE ring carries every load in exact consumption
    order (multiple rings round-robin at packet granularity and dilute
    critical early transfers behind bulk; a single FIFO ring measured
    ~390 GB/s of SBUF-write in v1). All tiles are dedicated (full
    prefetch run-ahead, no pool-slot throttling).
  - 8 groups land as bf16 via cast-DMA (write 1.05MB each); 8 land as
    fp8 slabs (write 0.52MB) and ScalarE pair-casts them — its 2x
    expansion happens off-fabric, balancing fabric (~15MB) vs the ACT
    chain (~28us) vs DVE (~39us).
  - DVE runs every dequant multiply at 2x bf16, pair-fused [128,2,4096]
    against a stride-0-broadcast wsT; x dequant is one broadcast TT.
    GpSimd runs zero tensor ops (shared SBUF port would halve both).
  - g0 chunk-granular at the head (halved DMAs + split wsT), g15
    chunk-granular at the tail (its data lands mid-stream, no cast).

Host-side marshaling stays format-only (LUT decode + layout + dtype cast);
all reference arithmetic (scale multiplies, matmul, bias) is on device.
"""

import json
from contextlib import ExitStack

import ml_dtypes
import numpy as np

import concourse.bass as bass
import concourse.mybir as mybir
import concourse.tile as tile
from concourse.bass_utils import run_bass_kernel_spmd


def _split_multi_waits(m: dict) -> dict:
    """This walrus build allows at most one sync-wait command per instruction.
    Hoist extra waits into standalone EventSemaphore instructions issued just
    before the owning instruction on the same engine queue (semantically
    identical: the engine stalls in order)."""
    for fn in m["functions"]:
        for blk in fn["blocks"]:
            new = []
            ctr = 0
            for inst in blk["instructions"]:
                si = inst.get("sync_info")
                waits = (si or {}).get("on_wait") or []
                if len(waits) > 1:
                    for w in waits[:-1]:
                        new.append({
                            "debug": inst.get("debug", 0),
                            "engine": inst["engine"],
                            "ins": [],
                            "outs": [],
                            "name": f"{inst['name']}-hw{ctr}",
                            "opcode": "EventSemaphore",
                            "sync_info": {"on_update": [], "on_wait": [w]},
                        })
                        ctr += 1
                    si["on_wait"] = [waits[-1]]
                new.append(inst)
            blk["instructions"] = new
    return m


class _SplitWaitBass(bass.Bass):
    def to_json_bytes(self) -> bytes:
        m = json.loads(super().to_json_bytes())
        return json.dumps(_split_multi_waits(m)).encode()


BF16 = ml_dtypes.bfloat16
FP4_LUT = np.array(
    [0.0, 0.5, 1.0, 1.5, 2.0, 3.0, 4.0, 6.0,
     -0.0, -0.5, -1.0, -1.5, -2.0, -3.0, -4.0, -6.0],
    dtype=np.float32,
)

M, K, N = 64, 8192, 8192
NCORES = 8
NS = N // NCORES        # 1024 output columns per core
BLOCK = 16
B = K // BLOCK          # 512 scale blocks along K
P = 128                 # partitions
CHUNKS = K // P         # 64 K-chunks
CB = B // P             # 4 scale-chunk columns (c index)
J = BLOCK               # 16 j-groups (one group = CB chunks = 512 rows)
GW = CB * NS            # 4096 columns per weight group tile
NQ = NS                 # 1024 columns per chunk of a group tile

# Host wvp layout positions (group order in DRAM):
#   [0 | 1,3 | 2,4 | 5,7 | 6,8 | 9,11 | 10,12 | 13,14 | 15]
# S (cast-DMA bf16): 0, (1,3), (5,7), (9,11), 15; A (fp8+ACT cast):
# (2,4), (6,8), (10,12), (13,14).
NEWORD = (0, 1, 3, 2, 4, 5, 7, 6, 8, 9, 11, 10, 12, 13, 14, 15)
POS = {g: i for i, g in enumerate(NEWORD)}
S_PAIRS = ((1, 3), (5, 7), (9, 11))
A_PAIRS = ((2, 4), (6, 8), (10, 12), (13, 14))

_CACHE: dict = {}


def _build_program() -> bass.Bass:
    nc = _SplitWaitBass("TRN2", target_bir_lowering=False, debug=False,
                        num_devices=NCORES)
    dt = mybir.dt

    wvp = nc.dram_tensor("wvp", [P, CHUNKS * NS], dt.float8e4,
                         kind="ExternalInput").ap()
    wst = nc.dram_tensor("wst", [P, GW], dt.bfloat16,
                         kind="ExternalInput").ap()
    xvp = nc.dram_tensor("xvp", [P, CHUNKS * M], dt.bfloat16,
                         kind="ExternalInput").ap()
    xst = nc.dram_tensor("xst", [P, CB * M], dt.bfloat16,
                         kind="ExternalInput").ap()
    gs = nc.dram_tensor("gs", [P, 2], dt.float32, kind="ExternalInput").ap()
    bia = nc.dram_tensor("bia", [1, NS], dt.bfloat16, kind="ExternalInput").ap()
    out = nc.dram_tensor("out", [M, NS], dt.bfloat16, kind="ExternalOutput").ap()

    def slab(g, n=1):
        return wvp[:, POS[g] * GW:(POS[g] + n) * GW]

    with tile.TileContext(nc) as tc, ExitStack() as ctx:
        const = ctx.enter_context(tc.tile_pool(name="const", bufs=1))
        w8pool = ctx.enter_context(tc.tile_pool(name="w8", bufs=1))
        whpool = ctx.enter_context(tc.tile_pool(name="wh", bufs=1))
        ppool = ctx.enter_context(tc.tile_pool(name="acc", bufs=1, space="PSUM"))

        wsT = const.tile([P, GW], dt.bfloat16)
        xva = const.tile([P, CHUNKS * M], dt.bfloat16)
        gt = const.tile([P, 2], dt.float32)
        xsT = const.tile([P, CB * M], dt.bfloat16)
        bsb = const.tile([1, NS], dt.bfloat16)
        whS0 = whpool.tile([P, GW], dt.bfloat16, name="whS0")
        whS15 = whpool.tile([P, GW], dt.bfloat16, name="whS15")
        ptS: dict = {}
        for a, b in S_PAIRS:
            ptS[a] = whpool.tile([P, 2 * GW], dt.bfloat16, name=f"ptS{a}")
        ptA: dict = {}
        w8: dict = {}
        for a, b in A_PAIRS:
            ptA[a] = whpool.tile([P, 2 * GW], dt.bfloat16, name=f"ptA{a}")
            w8[a] = w8pool.tile([P, 2 * GW], dt.float8e4, name=f"w8_{a}")

        # ---- head loads on the idle scalar/HWDGE ring (first issue ~1us
        # earlier than SWDGE and off the main stream), rest on SWDGE ----
        H = 2 * NQ
        nc.scalar.dma_start(wsT[:, 0:H], wst[:, 0:H])            # wsT half 0
        nc.gpsimd.dma_start(whS0[:, 0:H], wvp[:, 0:H])           # g0 half 0
        nc.gpsimd.dma_start(whS0[:, H:GW], wvp[:, H:GW])         # g0 half 1
        nc.gpsimd.dma_start(w8[2][:], slab(2, 2))                # A slab (2,4)
        nc.gpsimd.dma_start(wsT[:, H:GW], wst[:, H:GW])          # wsT half 1
        nc.gpsimd.dma_start(gt[:], gs[:])
        nc.gpsimd.dma_start(xsT[:], xst[:])
        nc.gpsimd.dma_start(xva[:], xvp[:])
        nc.gpsimd.dma_start(bsb[:], bia[:])
        nc.gpsimd.dma_start(w8[6][:], slab(6, 2))                # A slab (6,8)
        nc.gpsimd.dma_start(ptS[1][:], slab(1, 2))               # S pair (1,3)
        nc.gpsimd.dma_start(w8[10][:], slab(10, 2))              # A slab (10,12)
        nc.gpsimd.dma_start(ptS[5][:], slab(5, 2))               # S pair (5,7)
        nc.gpsimd.dma_start(w8[13][:], slab(13, 2))              # A slab (13,14)
        nc.gpsimd.dma_start(ptS[9][:], slab(9, 2))               # S pair (9,11)
        nc.gpsimd.dma_start(whS15[:], slab(15, 1))               # g15

        psum = ppool.tile([M, NS], dt.float32)
        ones = const.tile([1, M], dt.bfloat16)
        gcol = const.tile([P, 1], dt.float32)
        xsb = const.tile([P, CB * M], dt.bfloat16)
        xhat = const.tile([P, CHUNKS * M], dt.bfloat16)

        def emit_mms(g, c, wh, col0, stop=False):
            t = g * CB + c
            for h in range(2):
                nc.tensor.matmul(
                    psum[:, h * 512:(h + 1) * 512],
                    xhat[:, t * M:(t + 1) * M],
                    wh[:, col0 + c * NS + h * 512: col0 + c * NS + (h + 1) * 512],
                    start=False,
                    stop=stop,
                )

        wsT_b2 = wsT[:].unsqueeze(1).broadcast_to([P, 2, GW])

        def pair_tt(tle):
            nc.vector.tensor_mul(
                tle[:].rearrange("p (q w) -> p q w", q=2),
                tle[:].rearrange("p (q w) -> p q w", q=2),
                wsT_b2,
            )

        # ---- g0 chunk TTs (head; c0/c1 gated on half 0) ----
        for c in range(CB):
            cs = slice(c * NQ, (c + 1) * NQ)
            nc.vector.tensor_mul(whS0[:, cs], whS0[:, cs], wsT[:, cs])

        # ---- x-side ----
        nc.vector.tensor_mul(gcol[:], gt[:, 0:1], gt[:, 1:2])
        nc.vector.tensor_scalar_mul(xsb[:], xsT[:], gcol[:])
        xsb_b = xsb[:].unsqueeze(1).broadcast_to([P, J, CB * M])
        nc.vector.tensor_mul(
            xhat[:].rearrange("p (j w) -> p j w", j=J),
            xva[:].rearrange("p (j w) -> p j w", j=J),
            xsb_b,
        )
        nc.vector.memset(ones[:], 1.0)

        for h in range(2):
            nc.tensor.matmul(
                psum[:, h * 512:(h + 1) * 512],
                ones[:1, :],
                bsb[:1, h * 512:(h + 1) * 512],
                start=True,
                stop=False,
            )
        for c in range(CB):
            emit_mms(0, c, whS0, 0)

        # ---- main line: A/S pairs interleaved by arrival ----
        def do_pair(a, b, tle, is_a):
            if is_a:
                nc.scalar.copy(tle[:], w8[a][:])     # fused 2-group cast
            pair_tt(tle)
            for c in range(CB):
                emit_mms(a, c, tle, 0)
            for c in range(CB):
                emit_mms(b, c, tle, GW)

        do_pair(2, 4, ptA[2], True)
        do_pair(1, 3, ptS[1], False)
        do_pair(6, 8, ptA[6], True)
        do_pair(5, 7, ptS[5], False)
        do_pair(10, 12, ptA[10], True)
        do_pair(9, 11, ptS[9], False)
        do_pair(13, 14, ptA[13], True)

        # ---- g15 chunk TTs (tail; data landed mid-stream, no cast) ----
        for c in range(CB):
            cs = slice(c * NQ, (c + 1) * NQ)
            nc.vector.tensor_mul(whS15[:, cs], whS15[:, cs], wsT[:, cs])
            emit_mms(15, c, whS15, 0, stop=(c == CB - 1))

        # ---- tail ----
        osb = const.tile([M, NS], dt.bfloat16)
        nc.vector.tensor_copy(osb[:, 0:512], psum[:, 0:512])
        nc.scalar.copy(osb[:, 512:NS], psum[:, 512:NS])
        nc.sync.dma_start(out[:, 0:512], osb[:, 0:512])
        nc.scalar.dma_start(out[:, 512:NS], osb[:, 512:NS])

    return nc


def _perm_k(vals_2d: np.ndarray) -> np.ndarray:
    """[R, K] fp values -> [K, R] with K permuted as r = j*B + b."""
    r = vals_2d.shape[0]
    return (
        vals_2d.reshape(r, B, BLOCK).transpose(2, 1, 0).reshape(K, r)
    )


def _swz(rows_2d: np.ndarray, width: int) -> np.ndarray:
    """[n_chunks*128, width] -> [128, n_chunks*width]: row p holds chunk-major
    data for partition p (per-partition-contiguous DMA layout)."""
    n = rows_2d.shape[0] // P
    return np.ascontiguousarray(
        rows_2d.reshape(n, P, width).transpose(1, 0, 2).reshape(P, n * width)
    )


def prepare_in_maps(**inputs) -> list[dict[str, np.ndarray]]:
    x = np.asarray(inputs["x"]).astype(np.int64)
    xs = np.asarray(inputs["x_scale"], dtype=np.float32)
    gx = np.float32(np.asarray(inputs["x_global_scale"]).reshape(-1)[0])
    w = np.asarray(inputs["weight"]).astype(np.int64)
    ws = np.asarray(inputs["weight_scale"], dtype=np.float32)
    gw = np.float32(np.asarray(inputs["weight_global_scale"]).reshape(-1)[0])
    b = np.asarray(inputs["bias"], dtype=np.float32)

    FP8 = ml_dtypes.float8_e4m3
    xvp = _swz(_perm_k(FP4_LUT[x]).astype(BF16), M)                  # [128, 4096]
    xst = _swz(np.ascontiguousarray(xs.T), M).astype(BF16)           # [128, 256]
    gs = np.tile(np.array([[gx, gw]], dtype=np.float32), (P, 1))

    wv = FP4_LUT[w]                                                  # [N, K] f32
    in_maps = []
    for c in range(NCORES):
        sl = slice(c * NS, (c + 1) * NS)
        wvp = _swz(_perm_k(wv[sl]).astype(FP8), NS)                  # [128, 64*NS]
        wg = wvp.reshape(P, J, GW)[:, list(NEWORD), :]
        in_maps.append({
            "wvp": np.ascontiguousarray(wg.reshape(P, J * GW)),
            "wst": _swz(ws[sl].T.astype(BF16), NS),                  # [128, 4*NS]
            "xvp": xvp,
            "xst": xst,
            "gs": gs,
            "bia": np.ascontiguousarray(b[sl].reshape(1, NS)).astype(BF16),
        })
    return in_maps


LAST_RESULTS = None


def kernel(**inputs) -> np.ndarray:
    global LAST_RESULTS
    if "nc" not in _CACHE:
        _CACHE["nc"] = _build_program()
    nc = _CACHE["nc"]

    in_maps = prepare_in_maps(**inputs)
    res = run_bass_kernel_spmd(nc, in_maps, core_ids=list(range(NCORES)))
    LAST_RESULTS = res
    out = np.concatenate([res.results[c]["out"] for c in range(NCORES)], axis=1)
    return out.astype(BF16)
